# revision 80
# baseline (speedup 1.0000x reference)
"""Causal self-attention (softmax over the QUERY axis) for Trainium2, 8 cores.

Reference semantics (B=2, S=2048, D=1024, H=16, HD=64):
    q = x @ Wq; k = x @ Wk; v = x @ Wv          (per batch)
    s[b,h,q,k] = <q_bqh, k_bkh>;  mask k > q -> -inf
    w = softmax(s / sqrt(1024), axis=q)          # normalize over QUERY axis
    ctx[b,q,h,:] = sum_k w[b,h,q,k] * v[b,k,h,:]

Sharding: core c handles batch b = c // 4 and head group g = c % 4
(4 heads: 4g..4g+3).  Per core everything is done in a transposed
score layout S^T[k, q], which makes the query-axis softmax a FREE-AXIS
reduction, and the 1/Z[k] normalizer folds into V rows (no per-element
divide): ctx[q,d] = sum_k exp(s)/Z[k] * v[k,d] = sum_k exp(s) * (v[k,d]/Z[k]).

Structure:
  - Score rows for the two heads of a pair are emitted interleaved per
    512-col subchunk: head even lives in SBUF partitions 0-63 (PE row
    tile T0), head odd in 64-127 (T8), with separate PSUM pools, so the
    two matmul streams execute concurrently on the row-tiled PE array
    (~2x on the K=64 score matmuls).
  - Causal diag handling: ALL rows are masked post-exp on gpsimd
    (off the PE->ACT critical path).  For kt<8 the ACT accum includes
    the masked diag values, so gpsimd spills that triangle and a short
    negated 128-col DVE reduce subtracts its sum from Z (the masked
    part is <12% of Z for kt<8, so the bf16 round-trip is negligible;
    for kt>=8 it is NOT - those rows use a full-row DVE reduce).
  - Score PSUM is a shared 3-deep ring of [128,1024] tiles (6 banks),
    decoupling the PE from the exp ladder; rows kt<8 are chunked as
    (1024 | tail), the tail exp'd on DVE with a Schraudolph int16/bf16
    bit trick (those columns carry <10% of ctx mass, ~2% approx err),
    relieving the ACT engine.  Z: ACT accum_out for kt<8 main chunks
    (accumulator READS are expensive - never add more), DVE row reduce
    for kt>=8 and tails.
  - V is projected ONCE and kept pristine; 1/Z is applied into a small
    per-pair scaled copy v_s (bufs=2), so no V re-projection is needed
    for the second head pair.  V-tile, qk-proj and ctx output copies
    all run on ACT (scalar.copy) - DVE is the loaded engine.
  - ctx chunks are split into per-4-kt-row PIECES dripped one per score
    row through phase 1, so the PE never idles long enough to drop its
    DVFS p-state (idle => ~3us at half clock).  Emission-order
    invariants are load-bearing: a tile's writer must be emitted before
    any reader, and ctx pieces reading single-buffered e-rows must be
    emitted before pair 1's reuse of those rows.
  - Host pre-packs all inputs into exact SBUF layouts so every DMA is
    multi-KB contiguous runs; two issue queues (sync/scalar) ordered by
    first use; the triangle mask const is built on-device.
  - fp8e4m3 DoubleRow projections were tried and are numerically fine
    (rel err 1.5%) but NOT faster here: the kernel is exp/Z-ladder
    bound, and freeing PE time only deepens its p-state idling.
"""

import numpy as np
import ml_dtypes
from contextlib import ExitStack

import concourse.bass as bass
import concourse.tile as tile
from concourse import bacc, mybir
from concourse.bass_utils import run_bass_kernel_spmd

BF16 = mybir.dt.bfloat16
F32 = mybir.dt.float32
I16 = mybir.dt.int16
SCH_A = float((1.0 / 32.0) * 128.0 * np.log2(np.e))
SCH_B = 16250.0

B, S, D, H, HD = 2, 2048, 1024, 16, 64
NCORES = 8
HL = 4                       # heads per core
KC = D // 128                # 8 contraction chunks
KT = S // 128                # 16 key tiles
SCALE = 1.0 / float(np.sqrt(np.float32(D)))   # 1/32
NEG = -1.0e30


def _emit(ctx: ExitStack, tc: tile.TileContext, out_ap, xT, wq, wk, wv):
    nc = tc.nc
    Exp = mybir.ActivationFunctionType.Exp
    X = mybir.AxisListType.X
    ADD = mybir.AluOpType.add
    MULT = mybir.AluOpType.mult

    consts = ctx.enter_context(tc.tile_pool(name="consts", bufs=1))
    qkp = ctx.enter_context(tc.tile_pool(name="qk", bufs=1))
    vp = ctx.enter_context(tc.tile_pool(name="v", bufs=1))
    vsp = ctx.enter_context(tc.tile_pool(name="vs", bufs=2))
    outp = ctx.enter_context(tc.tile_pool(name="outp", bufs=2))
    epool = ctx.enter_context(tc.tile_pool(name="e", bufs=2))
    zpool = ctx.enter_context(tc.tile_pool(name="z", bufs=4))
    # PSUM: score ring 3 x [128,1024] (6 banks) + small 2 x [128,512]
    sc_ps = ctx.enter_context(tc.tile_pool(name="sc_ps", bufs=3,
                                           space="PSUM"))
    small_ps = ctx.enter_context(tc.tile_pool(name="small_ps", bufs=2,
                                              space="PSUM"))

    # ---- loads: host pre-packs every tensor into its exact SBUF layout,
    # so every DMA below moves multi-KB contiguous runs per partition.
    # Strict single-writer tiles: every DMA writes its own tile.
    wqk_sb = {}
    for name in ("q", "k"):
        for pair in (0, 1):
            wqk_sb[(name, pair)] = consts.tile(
                [128, KC, 128], BF16, tag=f"w{name}{pair}",
                name=f"w{name}{pair}_sb")
    wv_sb = consts.tile([128, KC, HL * HD], BF16, tag="wv", name="wv_sb")
    tri_sb = consts.tile([128, 128], BF16, tag="tri", name="tri_sb")
    xT_r = xT.rearrange("p (sc c s) -> p sc c s", sc=4, c=KC)
    wq_r = wq.rearrange("p (pr c n) -> p pr c n", pr=2, c=KC)
    wk_r = wk.rearrange("p (pr c n) -> p pr c n", pr=2, c=KC)
    wv_r = wv.rearrange("p (c n) -> p c n", c=KC)
    xT_cs = [None] * 4
    xT3h = [None, None]
    for sc in range(4):
        if sc == 3:
            for hc in range(2):
                xT3h[hc] = consts.tile([128, 4, 512], BF16,
                                       tag=f"xT3h{hc}",
                                       name=f"xT3h{hc}_sb")
            continue
        xT_cs[sc] = consts.tile([128, KC, 512], BF16, tag=f"xT{sc}",
                                name=f"xT{sc}_sb")

    # two issue queues, per-queue order matches fill consumption order;
    # xT3 split so the first projection chain starts after its low half
    nc.sync.dma_start(out=wqk_sb[("q", 0)], in_=wq_r[:, 0])
    nc.sync.dma_start(out=wqk_sb[("k", 0)], in_=wk_r[:, 0])
    nc.sync.dma_start(out=wqk_sb[("q", 1)], in_=wq_r[:, 1])
    nc.sync.dma_start(out=wqk_sb[("k", 1)], in_=wk_r[:, 1])
    nc.sync.dma_start(out=xT_cs[2], in_=xT_r[:, 2])
    nc.sync.dma_start(out=xT_cs[1], in_=xT_r[:, 1])
    nc.scalar.dma_start(out=xT3h[0], in_=xT_r[:, 3, 0:4])
    nc.scalar.dma_start(out=xT3h[1], in_=xT_r[:, 3, 4:8])
    nc.scalar.dma_start(out=wv_sb, in_=wv_r)
    nc.scalar.dma_start(out=xT_cs[0], in_=xT_r[:, 0])

    # triangle mask built on-device: NEG strictly below the diagonal
    nc.gpsimd.memset(tri_sb, NEG)
    nc.gpsimd.affine_select(
        tri_sb, tri_sb, pattern=[[-1, 128]],
        compare_op=mybir.AluOpType.is_ge, fill=0.0,
        base=-1, channel_multiplier=1,
    )

    def xT_slice(c, lo, w):
        sc, o = divmod(lo, 512)
        assert o + w <= 512
        if sc == 3:
            return xT3h[c // 4][:, c % 4, o:o + w]
        return xT_cs[sc][:, c, o:o + w]

    qT_sb = qkp.tile([128, 2, S], BF16, tag="qT")
    kT_sb = qkp.tile([128, 2, S], BF16, tag="kT")
    v_sb = vp.tile([128, KT, HL * HD], BF16, tag="v")
    vs_sb = {}

    def vs_tile(pair):
        if pair not in vs_sb:
            vs_sb[pair] = vsp.tile([128, KT, 2 * HD], BF16, tag="vs",
                                   name=f"vs{pair}")
        return vs_sb[pair]

    def proj_chain(name, pair, qc):
        dst = qT_sb if name == "q" else kT_sb
        ps = small_ps.tile([128, 512], F32, tag="ps512", name="pps")
        for c in range(KC):
            rhs = (xT3h[c // 4][:, c % 4, :] if qc == 3
                   else xT_cs[qc][:, c, :])
            nc.tensor.matmul(
                ps,
                wqk_sb[(name, pair)][:, c, :],
                rhs,
                start=(c == 0), stop=(c == KC - 1),
            )
        nc.vector.tensor_copy(dst[:, pair, 512 * qc:512 * qc + 512], ps)

    def proj_v(st_range):
        # v natural layout: out partitions = s-within-tile, cols = 4 heads x 64
        for st in st_range:
            ps = small_ps.tile([128, HL * HD], F32, tag="ps512", name="pps")
            for c in range(KC):
                nc.tensor.matmul(
                    ps,
                    xT_slice(c, 128 * st, 128),
                    wv_sb[:, c, :],
                    start=(c == 0), stop=(c == KC - 1),
                )
            nc.scalar.copy(v_sb[:, st, :], ps)

    def alloc_pair(pair):
        sts = []
        for hh in (0, 1):
            zp = zpool.tile([128, KT, 3], F32, tag="zp",
                            name=f"zp{2 * pair + hh}")
            nc.vector.memset(zp, 0.0)
            sts.append({"zp": zp, "e": [None] * KT, "h": 2 * pair + hh,
                        "hh": hh})
        return sts[0], sts[1]

    def score_row_pair(sta, stb, kt):
        """scores^T row kt for a head pair, interleaved on PE tiles T0/T8."""
        pair = sta["h"] // 2
        q0k = 128 * kt
        W = S - q0k
        rows = {}
        for half, st in ((0, sta), (1, stb)):
            e_row = epool.tile([128, W], BF16, tag=f"E{kt}h{half}",
                               name=f"e{kt}h{half}",
                               bufs=(2 if kt < 6 else 1))
            st["e"][kt] = e_row
            rows[half] = e_row
        if kt < 8:
            chunks = [(0, 1024), (1024, W - 1024)]
        else:
            chunks = [(0, W)]
        for ci, (lo, w) in enumerate(chunks):
            pss = {0: sc_ps.tile([128, w], F32, tag="sc", name=f"sA{kt}"),
                   1: sc_ps.tile([128, w], F32, tag="sc", name=f"sB{kt}")}
            c0 = 0
            while c0 < w:
                c1 = min(w, c0 + 512)
                for half in (0, 1):
                    pb = 64 * half
                    nc.tensor.matmul(
                        pss[half][:, c0:c1],
                        kT_sb[pb:pb + 64, pair, q0k:q0k + 128],
                        qT_sb[pb:pb + 64, pair, q0k + lo + c0:q0k + lo + c1],
                        start=True, stop=True,
                    )
                c0 = c1
            for half, st in ((0, sta), (1, stb)):
                if ci == 1:
                    # far tail: Schraudolph exp on DVE (bf16 bit trick);
                    # these keys carry ~<10% of ctx mass, ~2% approx err
                    tail = rows[half][:, lo:lo + w].bitcast(I16)
                    nc.vector.tensor_scalar(
                        out=tail, in0=pss[half][:, 0:w],
                        scalar1=SCH_A, scalar2=SCH_B,
                        op0=MULT, op1=ADD,
                    )
                    nc.vector.tensor_reduce(
                        st["zp"][:, kt, 1:2],
                        rows[half][:, lo:lo + w],
                        axis=X, op=ADD,
                    )
                elif kt < 8:
                    nc.scalar.activation(
                        rows[half][:, lo:lo + w], pss[half][:, 0:w],
                        Exp, scale=SCALE,
                        accum_out=st["zp"][:, kt, ci:ci + 1],
                    )
                else:
                    nc.scalar.activation(
                        rows[half][:, lo:lo + w], pss[half][:, 0:w],
                        Exp, scale=SCALE,
                    )
        for half, st in ((0, sta), (1, stb)):
            diag = rows[half][:, 0:128]
            if kt < 8:
                # ACT accum summed the full (unmasked) chunk0, so spill
                # the to-be-masked triangle (j < p) and subtract its sum
                # (a 128-col reduce; masked part is <12% of Z for kt<8,
                # so the bf16 round-trip mismatch is negligible)
                spill = epool.tile([128, 128], BF16, tag="spl",
                                   name="spill", bufs=2)
                nc.gpsimd.affine_select(
                    spill, diag, pattern=[[-1, 128]],
                    compare_op=mybir.AluOpType.is_ge, fill=0.0,
                    base=-1, channel_multiplier=1,
                )
                nc.vector.tensor_reduce(
                    st["zp"][:, kt, 2:3], spill,
                    axis=X, op=ADD, negate=True,
                )
            # post-exp diag mask on gpsimd (keep j >= p)
            nc.gpsimd.affine_select(
                diag, diag, pattern=[[1, 128]],
                compare_op=mybir.AluOpType.is_ge, fill=0.0,
                base=0, channel_multiplier=-1,
            )
            if kt >= 8:
                nc.vector.tensor_reduce(
                    st["zp"][:, kt, 0:1], rows[half][:, 0:W],
                    axis=X, op=ADD,
                )

    def z_v2(sta, stb, g):
        """finalize Z for kt group g (both heads) -> 1/Z-scaled V in v_s."""
        pair = sta["h"] // 2
        k0 = 4 * g
        for hh, st in ((0, sta), (1, stb)):
            zp = st["zp"]
            zs = zpool.tile([128, 4], F32, tag="zs", name="zs")
            nc.vector.tensor_reduce(zs, zp[:, k0:k0 + 4, :], axis=X,
                                    op=ADD)
            zi = zpool.tile([128, 4], F32, tag="zi", name="zi")
            nc.vector.reciprocal(zi, zs)
            zia = zi[:, :]
            zi_bc = bass.AP(tensor=zia.tensor, offset=zia.offset,
                            ap=[zia.ap[0], zia.ap[1], [0, HD]])
            nc.vector.tensor_mul(
                vs_tile(pair)[:, k0:k0 + 4, HD * hh:HD * hh + HD],
                v_sb[:, k0:k0 + 4, HD * (2 * pair + hh):
                     HD * (2 * pair + hh) + HD],
                zi_bc,
            )

    def ctx_pair_part(sta, stb, qc, ps, k0, k1):
        """col-packed ctx chain piece (kt in [k0,k1)) for a head pair."""
        pair = sta["h"] // 2
        vs = vs_tile(pair)
        n_kt = 4 * qc + 4
        for kt in range(k0, k1):
            q0 = max(512 * qc, 128 * kt)
            w = 512 * qc + 512 - q0
            for half, st in ((0, sta), (1, stb)):
                rhs = st["e"][kt][:, q0 - 128 * kt:q0 - 128 * kt + w]
                nc.tensor.matmul(
                    ps[64 * half:64 * half + 64, q0 - 512 * qc:512],
                    vs[:, kt, HD * half:HD * half + HD],
                    rhs,
                    start=(kt == 0), stop=(kt == n_kt - 1),
                    tile_position=(0, 64 * half),
                    skip_group_check=True,
                )

    def z_v2_row(sta, stb, kt):
        """per-row variant of z_v2 for the final group: unblocks the
        tail ctx matmuls one row at a time."""
        pair = sta["h"] // 2
        for hh, st in ((0, sta), (1, stb)):
            zp = st["zp"]
            zs = zpool.tile([128, 1], F32, tag="zs", name="zs")
            nc.vector.tensor_reduce(zs, zp[:, kt:kt + 1, :], axis=X, op=ADD)
            zi = zpool.tile([128, 1], F32, tag="zi", name="zi")
            nc.vector.reciprocal(zi, zs)
            zia = zi[:, :]
            zi_bc = bass.AP(tensor=zia.tensor, offset=zia.offset,
                            ap=[zia.ap[0], [0, HD]])
            nc.vector.tensor_mul(
                vs_tile(pair)[:, kt, HD * hh:HD * hh + HD],
                v_sb[:, kt, HD * (2 * pair + hh):HD * (2 * pair + hh) + HD],
                zi_bc,
            )

    ctx_ps = {}

    def ctx_kts(sta, stb, qc, k0, k1, close=False):
        """ctx chunk qc restricted to kt in [k0,k1)."""
        pair = sta["h"] // 2
        key = (pair, qc)
        if key not in ctx_ps:
            ctx_ps[key] = small_ps.tile([128, 512], F32, tag="ps512",
                                        name="cpp")
        ps = ctx_ps[key]
        ctx_pair_part(sta, stb, qc, ps, k0, k1)
        if close:
            del ctx_ps[key]
            ot = outp.tile([128, 512], BF16, tag="ot", name="ot")
            nc.scalar.copy(ot, ps)
            nc.sync.dma_start(
                out=out_ap[128 * pair:128 * pair + 128,
                           512 * qc:512 * qc + 512],
                in_=ot,
            )

    def ctx_piece(sta, stb, qc, g, close=False):
        """ctx chunk qc, kt group g (4 kt rows) — spreads the chunk's
        accumulation over multiple fill slots for fine PE pacing."""
        ctx_kts(sta, stb, qc, 4 * g, min(4 * g + 4, 4 * qc + 4),
                close=close)

    def ctx_pair_packed(sta, stb, qc):
        for g in range(qc + 1):
            ctx_piece(sta, stb, qc, g, close=(g == qc))

    # ---- emission (order = scheduling priority) ----
    # Per pair: rows 15..8 descend (chunk-progressive proj, feeds ACT
    # early), then rows 0..7 ascend so the low (chain-gating) rows and
    # their per-group Z finish early and ctx chains overlap the tail.
    # PE filler is interleaved in drain-sized quanta.
    st0, st1 = alloc_pair(0)
    st2, st3 = alloc_pair(1)
    ROWS = [15, 14, 13, 12, 11, 10, 9, 8, 0, 1, 2, 3, 4, 5, 6, 7]

    def run_rows(sta, stb, rows, sched, fill, post):
        fi = 0
        for kt in rows:
            score_row_pair(sta, stb, kt)
            for _ in range(sched[kt]):
                if fi < len(fill):
                    fill[fi]()
                    fi += 1
            if kt in post:
                for f in post[kt]:
                    f()
        while fi < len(fill):
            fill[fi]()
            fi += 1

    proj_chain("q", 0, 3)
    proj_chain("k", 0, 3)
    # EMISSION-ORDER INVARIANT: a chain/v tile must be EMITTED (not just
    # data-ready) before any score row / z_v2 / ctx that reads it — the
    # tile framework cannot order a read emitted before its writer.
    # Also: ctx01(0..3) must be emitted before P1's first kt>=8 row
    # (its e-row buffer ring reuses P0's kt>=8 slots).
    #
    # Work is balanced PE<->ACT across both phases so the PE never goes
    # idle (idle drops the PE to its slow p-state for ~3us): phase 0 =
    # chains + P0 scores + v0..11; phase 1 = v12..15 + P1 scores + all
    # z/ctx/output work.
    fill0 = [lambda: proj_chain("q", 1, 3),
             lambda: proj_chain("k", 1, 3),
             lambda: proj_v([15]), lambda: proj_chain("q", 0, 2),
             lambda: proj_chain("k", 0, 2), lambda: proj_v([14]),
             lambda: proj_chain("q", 0, 1), lambda: proj_chain("k", 0, 1),
             lambda: proj_v([13]), lambda: proj_v([12]),
             lambda: proj_chain("q", 0, 0), lambda: proj_chain("k", 0, 0),
             lambda: proj_v([11]), lambda: proj_v([10]),
             lambda: proj_v([9]), lambda: proj_v([8]),
             lambda: proj_chain("q", 1, 2), lambda: proj_chain("k", 1, 2),
             lambda: proj_v([3]), lambda: proj_v([2]),
             lambda: proj_v([1]), lambda: proj_v([0]),
             lambda: proj_chain("q", 1, 1), lambda: proj_chain("k", 1, 1),
             lambda: proj_v([7]), lambda: proj_v([6]),
             lambda: proj_v([5]), lambda: proj_v([4]),
             lambda: proj_chain("q", 1, 0), lambda: proj_chain("k", 1, 0)]
    sched0 = {15: 1, 14: 1, 13: 2, 12: 2, 11: 2, 10: 2, 9: 2, 8: 2,
              0: 2, 1: 2, 2: 2, 3: 2, 4: 2, 5: 2, 6: 2, 7: 2}
    post0 = {3: [lambda: z_v2(st0, st1, 0)],
             8: [lambda: z_v2(st0, st1, 3)],
             0: [lambda: z_v2(st0, st1, 2)],
             7: [lambda: z_v2(st0, st1, 1)]}
    run_rows(st0, st1, ROWS, sched0, fill0, post0)

    # phase 1: P1 rows ascending; P0's ctx pieces drip in as PE filler
    # (one small piece per row) so the PE never idles long enough to
    # drop its p-state.  Ordering constraints: piece (qc,g) reading a
    # bufs=1 e-row group g must be emitted before P1's score row 4g;
    # at most two ctx chunks hold PSUM accumulators at once.
    fill1 = [lambda: ctx_piece(st0, st1, 0, 0, close=True),
             lambda: ctx_piece(st0, st1, 1, 0),
             lambda: ctx_piece(st0, st1, 1, 1, close=True),
             lambda: ctx_piece(st0, st1, 2, 0),
             lambda: ctx_piece(st0, st1, 2, 1),
             lambda: (ctx_piece(st0, st1, 2, 2, close=True),
                      ctx_piece(st0, st1, 3, 0),
                      ctx_piece(st0, st1, 3, 1)),
             lambda: ctx_piece(st0, st1, 3, 2),
             lambda: ctx_piece(st0, st1, 3, 3, close=True),
             lambda: ctx_piece(st2, st3, 2, 0),
             lambda: ctx_piece(st2, st3, 2, 1),
             lambda: ctx_piece(st2, st3, 3, 0),
             lambda: ctx_piece(st2, st3, 3, 1),
             lambda: ctx_piece(st2, st3, 3, 2)]
    sched1 = {0: 1, 1: 1, 2: 1, 3: 1, 4: 1, 5: 1, 6: 1, 7: 1,
              8: 0, 9: 1, 10: 1, 11: 0, 12: 1, 13: 1, 14: 1, 15: 0}
    post1 = {3: [lambda: z_v2(st2, st3, 0),
                 lambda: ctx_piece(st2, st3, 0, 0, close=True)],
             7: [lambda: z_v2(st2, st3, 1),
                 lambda: ctx_piece(st2, st3, 1, 0),
                 lambda: ctx_piece(st2, st3, 1, 1, close=True)],
             11: [lambda: z_v2(st2, st3, 2),
                  lambda: ctx_piece(st2, st3, 2, 2, close=True)],
             15: [lambda: z_v2(st2, st3, 3),
                  lambda: ctx_piece(st2, st3, 3, 3, close=True)]}
    ROWS1 = list(range(KT))
    run_rows(st2, st3, ROWS1, sched1, fill1, post1)


_PROG = None


def _build_program():
    global _PROG
    if _PROG is not None:
        return _PROG
    nc = bacc.Bacc("TRN2", target_bir_lowering=False, debug=False,
                   num_devices=NCORES)
    xT = nc.dram_tensor("xT", [128, 4 * KC * 512], BF16,
                        kind="ExternalInput").ap()
    wq = nc.dram_tensor("wq", [128, 2 * KC * 128], BF16,
                        kind="ExternalInput").ap()
    wk = nc.dram_tensor("wk", [128, 2 * KC * 128], BF16,
                        kind="ExternalInput").ap()
    wv = nc.dram_tensor("wv", [128, KC * HL * HD], BF16,
                        kind="ExternalInput").ap()
    out = nc.dram_tensor("out", [HL * HD, S], BF16, kind="ExternalOutput").ap()
    with tile.TileContext(nc) as tc:
        with ExitStack() as stack:
            _emit(stack, tc, out, xT, wq, wk, wv)
    nc.compile()
    _PROG = nc
    return nc


def _pack_x(xb):
    # x[b] [S, D] -> xT [D, S] -> [128, sc, c, 512]: row p holds, per
    # 512-query chunk sc, all KC contraction chunks contiguously.
    xT = np.asarray(xb).T                      # [D, S] = [c*128+p, s]
    t = xT.reshape(KC, 128, 4, 512)            # [c, p, sc, s]
    return np.ascontiguousarray(
        t.transpose(1, 2, 0, 3).reshape(128, 4 * KC * 512))


def _pack_wqk(W):
    # W [D, 256] -> [128, pair, c, 128]
    t = np.asarray(W).reshape(KC, 128, 2, 128)  # [c, p, pair, n]
    return np.ascontiguousarray(
        t.transpose(1, 2, 0, 3).reshape(128, 2 * KC * 128))


def _pack_wv(W):
    # W [D, 256] -> [128, c, 256]
    t = np.asarray(W).reshape(KC, 128, HL * HD)  # [c, p, n]
    return np.ascontiguousarray(
        t.transpose(1, 0, 2).reshape(128, KC * HL * HD))


def make_in_maps(x, Wq, Wk, Wv):
    bf = ml_dtypes.bfloat16
    in_maps = []
    for core in range(NCORES):
        b, g = divmod(core, NCORES // B)
        cols = slice(HL * HD * g, HL * HD * (g + 1))
        in_maps.append({
            "xT": _pack_x(x[b]).astype(bf),
            "wq": _pack_wqk(np.asarray(Wq)[:, cols]).astype(bf),
            "wk": _pack_wqk(np.asarray(Wk)[:, cols]).astype(bf),
            "wv": _pack_wv(np.asarray(Wv)[:, cols]).astype(bf),
        })
    return in_maps


def assemble(results):
    out = np.empty((B, S, H * HD), np.float32)
    for core in range(NCORES):
        b, g = divmod(core, NCORES // B)
        out[b, :, HL * HD * g:HL * HD * (g + 1)] = \
            results[core]["out"].astype(np.float32).T
    return out


def kernel(**inputs):
    nc = _build_program()
    in_maps = make_in_maps(inputs["x"], inputs["Wq"], inputs["Wk"], inputs["Wv"])
    res = run_bass_kernel_spmd(nc, in_maps, list(range(NCORES)))
    return assemble(res.results)


# revision 81
# speedup vs baseline: 1.0355x; 1.0355x over previous
"""Causal self-attention (softmax over the QUERY axis) for Trainium2, 8 cores.

Reference semantics (B=2, S=2048, D=1024, H=16, HD=64):
    q = x @ Wq; k = x @ Wk; v = x @ Wv          (per batch)
    s[b,h,q,k] = <q_bqh, k_bkh>;  mask k > q -> -inf
    w = softmax(s / sqrt(1024), axis=q)          # normalize over QUERY axis
    ctx[b,q,h,:] = sum_k w[b,h,q,k] * v[b,k,h,:]

Sharding: core c handles batch b = c // 4 and head group g = c % 4
(4 heads: 4g..4g+3).  Per core everything is done in a transposed
score layout S^T[k, q], which makes the query-axis softmax a FREE-AXIS
reduction, and the 1/Z[k] normalizer folds into V rows (no per-element
divide): ctx[q,d] = sum_k exp(s)/Z[k] * v[k,d] = sum_k exp(s) * (v[k,d]/Z[k]).

Structure:
  - Score rows for the two heads of a pair are emitted interleaved per
    512-col subchunk: head even lives in SBUF partitions 0-63 (PE row
    tile T0), head odd in 64-127 (T8), with separate PSUM pools, so the
    two matmul streams execute concurrently on the row-tiled PE array
    (~2x on the K=64 score matmuls).
  - Causal diag handling: ALL rows are masked post-exp on gpsimd
    (off the PE->ACT critical path).  For kt<8 the ACT accum includes
    the masked diag values, so gpsimd spills that triangle and a short
    negated 128-col DVE reduce subtracts its sum from Z (the masked
    part is <12% of Z for kt<8, so the bf16 round-trip is negligible;
    for kt>=8 it is NOT - those rows use a full-row DVE reduce).
  - Score PSUM is a shared 3-deep ring of [128,1024] tiles (6 banks),
    decoupling the PE from the exp ladder; rows kt<8 are chunked as
    (1024 | tail), the tail exp'd on DVE with a Schraudolph int16/bf16
    bit trick (those columns carry <10% of ctx mass, ~2% approx err),
    relieving the ACT engine.  Z: ACT accum_out for kt<8 main chunks
    (accumulator READS are expensive - never add more), DVE row reduce
    for kt>=8 and tails.
  - V is projected ONCE and kept pristine; 1/Z is applied into a small
    per-pair scaled copy v_s (bufs=2), so no V re-projection is needed
    for the second head pair.  V-tile, qk-proj and ctx output copies
    all run on ACT (scalar.copy) - DVE is the loaded engine.
  - ctx chunks are split into per-4-kt-row PIECES dripped one per score
    row through phase 1, so the PE never idles long enough to drop its
    DVFS p-state (idle => ~3us at half clock).  Emission-order
    invariants are load-bearing: a tile's writer must be emitted before
    any reader, and ctx pieces reading single-buffered e-rows must be
    emitted before pair 1's reuse of those rows.
  - Host pre-packs all inputs into exact SBUF layouts so every DMA is
    multi-KB contiguous runs; two issue queues (sync/scalar) ordered by
    first use; the triangle mask const is built on-device.
  - fp8e4m3 DoubleRow projections were tried and are numerically fine
    (rel err 1.5%) but NOT faster here: the kernel is exp/Z-ladder
    bound, and freeing PE time only deepens its p-state idling.
"""

import numpy as np
import ml_dtypes
from contextlib import ExitStack

import concourse.bass as bass
import concourse.tile as tile
from concourse import bacc, mybir
from concourse.bass_utils import run_bass_kernel_spmd

BF16 = mybir.dt.bfloat16
F32 = mybir.dt.float32
I16 = mybir.dt.int16
SCH_A = float((1.0 / 32.0) * 128.0 * np.log2(np.e))
SCH_B = 16250.0

B, S, D, H, HD = 2, 2048, 1024, 16, 64
NCORES = 8
HL = 4                       # heads per core
KC = D // 128                # 8 contraction chunks
KT = S // 128                # 16 key tiles
SCALE = 1.0 / float(np.sqrt(np.float32(D)))   # 1/32
NEG = -1.0e30


def _emit(ctx: ExitStack, tc: tile.TileContext, out_ap, xT, wq, wk, wv):
    nc = tc.nc
    Exp = mybir.ActivationFunctionType.Exp
    X = mybir.AxisListType.X
    ADD = mybir.AluOpType.add
    MULT = mybir.AluOpType.mult

    consts = ctx.enter_context(tc.tile_pool(name="consts", bufs=1))
    qkp = ctx.enter_context(tc.tile_pool(name="qk", bufs=1))
    vp = ctx.enter_context(tc.tile_pool(name="v", bufs=1))
    vsp = ctx.enter_context(tc.tile_pool(name="vs", bufs=2))
    outp = ctx.enter_context(tc.tile_pool(name="outp", bufs=2))
    epool = ctx.enter_context(tc.tile_pool(name="e", bufs=2))
    zpool = ctx.enter_context(tc.tile_pool(name="z", bufs=4))
    # PSUM: score ring 3 x [128,1024] (6 banks) + small 2 x [128,512]
    sc_ps = ctx.enter_context(tc.tile_pool(name="sc_ps", bufs=3,
                                           space="PSUM"))
    small_ps = ctx.enter_context(tc.tile_pool(name="small_ps", bufs=2,
                                              space="PSUM"))

    # ---- loads: host pre-packs every tensor into its exact SBUF layout,
    # so every DMA below moves multi-KB contiguous runs per partition.
    # Strict single-writer tiles: every DMA writes its own tile.
    wqk_sb = {}
    for name in ("q", "k"):
        for pair in (0, 1):
            wqk_sb[(name, pair)] = consts.tile(
                [128, KC, 128], BF16, tag=f"w{name}{pair}",
                name=f"w{name}{pair}_sb")
    wv_sb = consts.tile([128, KC, HL * HD], BF16, tag="wv", name="wv_sb")
    tri_sb = consts.tile([128, 128], BF16, tag="tri", name="tri_sb")
    xT_r = xT.rearrange("p (sc c s) -> p sc c s", sc=4, c=KC)
    wq_r = wq.rearrange("p (pr c n) -> p pr c n", pr=2, c=KC)
    wk_r = wk.rearrange("p (pr c n) -> p pr c n", pr=2, c=KC)
    wv_r = wv.rearrange("p (c n) -> p c n", c=KC)
    xT_cs = [None] * 4
    xT3h = [None, None]
    for sc in range(4):
        if sc == 3:
            for hc in range(2):
                xT3h[hc] = consts.tile([128, 4, 512], BF16,
                                       tag=f"xT3h{hc}",
                                       name=f"xT3h{hc}_sb")
            continue
        xT_cs[sc] = consts.tile([128, KC, 512], BF16, tag=f"xT{sc}",
                                name=f"xT{sc}_sb")

    # two issue queues, per-queue order matches fill consumption order;
    # xT3 split so the first projection chain starts after its low half
    nc.sync.dma_start(out=wqk_sb[("q", 0)], in_=wq_r[:, 0])
    nc.sync.dma_start(out=wqk_sb[("k", 0)], in_=wk_r[:, 0])
    nc.sync.dma_start(out=wqk_sb[("q", 1)], in_=wq_r[:, 1])
    nc.sync.dma_start(out=wqk_sb[("k", 1)], in_=wk_r[:, 1])
    nc.sync.dma_start(out=xT_cs[2], in_=xT_r[:, 2])
    nc.sync.dma_start(out=xT_cs[1], in_=xT_r[:, 1])
    nc.scalar.dma_start(out=xT3h[0], in_=xT_r[:, 3, 0:4])
    nc.scalar.dma_start(out=xT3h[1], in_=xT_r[:, 3, 4:8])
    nc.scalar.dma_start(out=wv_sb, in_=wv_r)
    nc.scalar.dma_start(out=xT_cs[0], in_=xT_r[:, 0])

    # triangle mask built on-device: NEG strictly below the diagonal
    nc.gpsimd.memset(tri_sb, NEG)
    nc.gpsimd.affine_select(
        tri_sb, tri_sb, pattern=[[-1, 128]],
        compare_op=mybir.AluOpType.is_ge, fill=0.0,
        base=-1, channel_multiplier=1,
    )

    def xT_slice(c, lo, w):
        sc, o = divmod(lo, 512)
        assert o + w <= 512
        if sc == 3:
            return xT3h[c // 4][:, c % 4, o:o + w]
        return xT_cs[sc][:, c, o:o + w]

    qT_sb = qkp.tile([128, 2, S], BF16, tag="qT")
    kT_sb = qkp.tile([128, 2, S], BF16, tag="kT")
    v_sb = vp.tile([128, KT, HL * HD], BF16, tag="v")
    vs_sb = {}

    def vs_tile(pair):
        if pair not in vs_sb:
            vs_sb[pair] = vsp.tile([128, KT, 2 * HD], BF16, tag="vs",
                                   name=f"vs{pair}")
        return vs_sb[pair]

    def proj_chain(name, pair, qc):
        dst = qT_sb if name == "q" else kT_sb
        ps = small_ps.tile([128, 512], F32, tag="ps512", name="pps")
        for c in range(KC):
            rhs = (xT3h[c // 4][:, c % 4, :] if qc == 3
                   else xT_cs[qc][:, c, :])
            nc.tensor.matmul(
                ps,
                wqk_sb[(name, pair)][:, c, :],
                rhs,
                start=(c == 0), stop=(c == KC - 1),
            )
        nc.scalar.copy(dst[:, pair, 512 * qc:512 * qc + 512], ps)

    def proj_v(st_range):
        # v natural layout: out partitions = s-within-tile, cols = 4 heads x 64
        for st in st_range:
            ps = small_ps.tile([128, HL * HD], F32, tag="ps512", name="pps")
            for c in range(KC):
                nc.tensor.matmul(
                    ps,
                    xT_slice(c, 128 * st, 128),
                    wv_sb[:, c, :],
                    start=(c == 0), stop=(c == KC - 1),
                )
            nc.scalar.copy(v_sb[:, st, :], ps)

    def alloc_pair(pair):
        sts = []
        for hh in (0, 1):
            zp = zpool.tile([128, KT, 3], F32, tag="zp",
                            name=f"zp{2 * pair + hh}")
            nc.vector.memset(zp, 0.0)
            sts.append({"zp": zp, "e": [None] * KT, "h": 2 * pair + hh,
                        "hh": hh})
        return sts[0], sts[1]

    def score_row_pair(sta, stb, kt):
        """scores^T row kt for a head pair, interleaved on PE tiles T0/T8."""
        pair = sta["h"] // 2
        q0k = 128 * kt
        W = S - q0k
        rows = {}
        for half, st in ((0, sta), (1, stb)):
            e_row = epool.tile([128, W], BF16, tag=f"E{kt}h{half}",
                               name=f"e{kt}h{half}",
                               bufs=(2 if kt < 6 else 1))
            st["e"][kt] = e_row
            rows[half] = e_row
        if kt < 8:
            chunks = [(0, 1024), (1024, W - 1024)]
        else:
            chunks = [(0, W)]
        for ci, (lo, w) in enumerate(chunks):
            pss = {0: sc_ps.tile([128, w], F32, tag="sc", name=f"sA{kt}"),
                   1: sc_ps.tile([128, w], F32, tag="sc", name=f"sB{kt}")}
            c0 = 0
            while c0 < w:
                c1 = min(w, c0 + 512)
                for half in (0, 1):
                    pb = 64 * half
                    nc.tensor.matmul(
                        pss[half][:, c0:c1],
                        kT_sb[pb:pb + 64, pair, q0k:q0k + 128],
                        qT_sb[pb:pb + 64, pair, q0k + lo + c0:q0k + lo + c1],
                        start=True, stop=True,
                    )
                c0 = c1
            for half, st in ((0, sta), (1, stb)):
                if ci == 1:
                    # far tail: Schraudolph exp on DVE (bf16 bit trick);
                    # these keys carry ~<10% of ctx mass, ~2% approx err
                    tail = rows[half][:, lo:lo + w].bitcast(I16)
                    nc.vector.tensor_scalar(
                        out=tail, in0=pss[half][:, 0:w],
                        scalar1=SCH_A, scalar2=SCH_B,
                        op0=MULT, op1=ADD,
                    )
                    nc.vector.tensor_reduce(
                        st["zp"][:, kt, 1:2],
                        rows[half][:, lo:lo + w],
                        axis=X, op=ADD,
                    )
                elif kt < 8:
                    nc.scalar.activation(
                        rows[half][:, lo:lo + w], pss[half][:, 0:w],
                        Exp, scale=SCALE,
                        accum_out=st["zp"][:, kt, ci:ci + 1],
                    )
                else:
                    nc.scalar.activation(
                        rows[half][:, lo:lo + w], pss[half][:, 0:w],
                        Exp, scale=SCALE,
                    )
        for half, st in ((0, sta), (1, stb)):
            diag = rows[half][:, 0:128]
            if kt < 8:
                # ACT accum summed the full (unmasked) chunk0, so spill
                # the to-be-masked triangle (j < p) and subtract its sum
                # (a 128-col reduce; masked part is <12% of Z for kt<8,
                # so the bf16 round-trip mismatch is negligible)
                spill = epool.tile([128, 128], BF16, tag="spl",
                                   name="spill", bufs=2)
                nc.gpsimd.affine_select(
                    spill, diag, pattern=[[-1, 128]],
                    compare_op=mybir.AluOpType.is_ge, fill=0.0,
                    base=-1, channel_multiplier=1,
                )
                nc.vector.tensor_reduce(
                    st["zp"][:, kt, 2:3], spill,
                    axis=X, op=ADD, negate=True,
                )
            # post-exp diag mask on gpsimd (keep j >= p)
            nc.gpsimd.affine_select(
                diag, diag, pattern=[[1, 128]],
                compare_op=mybir.AluOpType.is_ge, fill=0.0,
                base=0, channel_multiplier=-1,
            )
            if kt >= 8:
                nc.vector.tensor_reduce(
                    st["zp"][:, kt, 0:1], rows[half][:, 0:W],
                    axis=X, op=ADD,
                )

    def z_v2(sta, stb, g):
        """finalize Z for kt group g (both heads) -> 1/Z-scaled V in v_s."""
        pair = sta["h"] // 2
        k0 = 4 * g
        for hh, st in ((0, sta), (1, stb)):
            zp = st["zp"]
            zs = zpool.tile([128, 4], F32, tag="zs", name="zs")
            nc.vector.tensor_reduce(zs, zp[:, k0:k0 + 4, :], axis=X,
                                    op=ADD)
            zi = zpool.tile([128, 4], F32, tag="zi", name="zi")
            nc.vector.reciprocal(zi, zs)
            zia = zi[:, :]
            zi_bc = bass.AP(tensor=zia.tensor, offset=zia.offset,
                            ap=[zia.ap[0], zia.ap[1], [0, HD]])
            nc.vector.tensor_mul(
                vs_tile(pair)[:, k0:k0 + 4, HD * hh:HD * hh + HD],
                v_sb[:, k0:k0 + 4, HD * (2 * pair + hh):
                     HD * (2 * pair + hh) + HD],
                zi_bc,
            )

    def ctx_pair_part(sta, stb, qc, ps, k0, k1):
        """col-packed ctx chain piece (kt in [k0,k1)) for a head pair."""
        pair = sta["h"] // 2
        vs = vs_tile(pair)
        n_kt = 4 * qc + 4
        for kt in range(k0, k1):
            q0 = max(512 * qc, 128 * kt)
            w = 512 * qc + 512 - q0
            for half, st in ((0, sta), (1, stb)):
                rhs = st["e"][kt][:, q0 - 128 * kt:q0 - 128 * kt + w]
                nc.tensor.matmul(
                    ps[64 * half:64 * half + 64, q0 - 512 * qc:512],
                    vs[:, kt, HD * half:HD * half + HD],
                    rhs,
                    start=(kt == 0), stop=(kt == n_kt - 1),
                    tile_position=(0, 64 * half),
                    skip_group_check=True,
                )

    def z_v2_row(sta, stb, kt):
        """per-row variant of z_v2 for the final group: unblocks the
        tail ctx matmuls one row at a time."""
        pair = sta["h"] // 2
        for hh, st in ((0, sta), (1, stb)):
            zp = st["zp"]
            zs = zpool.tile([128, 1], F32, tag="zs", name="zs")
            nc.vector.tensor_reduce(zs, zp[:, kt:kt + 1, :], axis=X, op=ADD)
            zi = zpool.tile([128, 1], F32, tag="zi", name="zi")
            nc.vector.reciprocal(zi, zs)
            zia = zi[:, :]
            zi_bc = bass.AP(tensor=zia.tensor, offset=zia.offset,
                            ap=[zia.ap[0], [0, HD]])
            nc.vector.tensor_mul(
                vs_tile(pair)[:, kt, HD * hh:HD * hh + HD],
                v_sb[:, kt, HD * (2 * pair + hh):HD * (2 * pair + hh) + HD],
                zi_bc,
            )

    ctx_ps = {}

    def ctx_kts(sta, stb, qc, k0, k1, close=False):
        """ctx chunk qc restricted to kt in [k0,k1)."""
        pair = sta["h"] // 2
        key = (pair, qc)
        if key not in ctx_ps:
            ctx_ps[key] = small_ps.tile([128, 512], F32, tag="ps512",
                                        name="cpp")
        ps = ctx_ps[key]
        ctx_pair_part(sta, stb, qc, ps, k0, k1)
        if close:
            del ctx_ps[key]
            ot = outp.tile([128, 512], BF16, tag="ot", name="ot")
            nc.scalar.copy(ot, ps)
            nc.sync.dma_start(
                out=out_ap[128 * pair:128 * pair + 128,
                           512 * qc:512 * qc + 512],
                in_=ot,
            )

    def ctx_piece(sta, stb, qc, g, close=False):
        """ctx chunk qc, kt group g (4 kt rows) — spreads the chunk's
        accumulation over multiple fill slots for fine PE pacing."""
        ctx_kts(sta, stb, qc, 4 * g, min(4 * g + 4, 4 * qc + 4),
                close=close)

    def ctx_pair_packed(sta, stb, qc):
        for g in range(qc + 1):
            ctx_piece(sta, stb, qc, g, close=(g == qc))

    # ---- emission (order = scheduling priority) ----
    # Per pair: rows 15..8 descend (chunk-progressive proj, feeds ACT
    # early), then rows 0..7 ascend so the low (chain-gating) rows and
    # their per-group Z finish early and ctx chains overlap the tail.
    # PE filler is interleaved in drain-sized quanta.
    st0, st1 = alloc_pair(0)
    st2, st3 = alloc_pair(1)
    ROWS = [15, 14, 13, 12, 11, 10, 9, 8, 0, 1, 2, 3, 4, 5, 6, 7]

    def run_rows(sta, stb, rows, sched, fill, post):
        fi = 0
        for kt in rows:
            score_row_pair(sta, stb, kt)
            for _ in range(sched[kt]):
                if fi < len(fill):
                    fill[fi]()
                    fi += 1
            if kt in post:
                for f in post[kt]:
                    f()
        while fi < len(fill):
            fill[fi]()
            fi += 1

    proj_chain("q", 0, 3)
    proj_chain("k", 0, 3)
    # EMISSION-ORDER INVARIANT: a chain/v tile must be EMITTED (not just
    # data-ready) before any score row / z_v2 / ctx that reads it — the
    # tile framework cannot order a read emitted before its writer.
    # Also: ctx01(0..3) must be emitted before P1's first kt>=8 row
    # (its e-row buffer ring reuses P0's kt>=8 slots).
    #
    # Work is balanced PE<->ACT across both phases so the PE never goes
    # idle (idle drops the PE to its slow p-state for ~3us): phase 0 =
    # chains + P0 scores + v0..11; phase 1 = v12..15 + P1 scores + all
    # z/ctx/output work.
    fill0 = [lambda: proj_chain("q", 1, 3),
             lambda: proj_chain("k", 1, 3),
             lambda: proj_v([15]), lambda: proj_chain("q", 0, 2),
             lambda: proj_chain("k", 0, 2), lambda: proj_v([14]),
             lambda: proj_chain("q", 0, 1), lambda: proj_chain("k", 0, 1),
             lambda: proj_v([13]), lambda: proj_v([12]),
             lambda: proj_chain("q", 0, 0), lambda: proj_chain("k", 0, 0),
             lambda: proj_v([11]), lambda: proj_v([10]),
             lambda: proj_v([9]), lambda: proj_v([8]),
             lambda: proj_chain("q", 1, 2), lambda: proj_chain("k", 1, 2),
             lambda: proj_v([3]), lambda: proj_v([2]),
             lambda: proj_v([1]), lambda: proj_v([0]),
             lambda: proj_chain("q", 1, 1), lambda: proj_chain("k", 1, 1),
             lambda: proj_v([7]), lambda: proj_v([6]),
             lambda: proj_v([5]), lambda: proj_v([4]),
             lambda: proj_chain("q", 1, 0), lambda: proj_chain("k", 1, 0)]
    sched0 = {15: 1, 14: 1, 13: 2, 12: 2, 11: 2, 10: 2, 9: 2, 8: 2,
              0: 2, 1: 2, 2: 2, 3: 2, 4: 2, 5: 2, 6: 2, 7: 2}
    post0 = {3: [lambda: z_v2(st0, st1, 0)],
             8: [lambda: z_v2(st0, st1, 3)],
             0: [lambda: z_v2(st0, st1, 2)],
             7: [lambda: z_v2(st0, st1, 1)]}
    run_rows(st0, st1, ROWS, sched0, fill0, post0)

    # phase 1: P1 rows ascending; P0's ctx pieces drip in as PE filler
    # (one small piece per row) so the PE never idles long enough to
    # drop its p-state.  Ordering constraints: piece (qc,g) reading a
    # bufs=1 e-row group g must be emitted before P1's score row 4g;
    # at most two ctx chunks hold PSUM accumulators at once.
    fill1 = [lambda: ctx_piece(st0, st1, 0, 0, close=True),
             lambda: ctx_piece(st0, st1, 1, 0),
             lambda: ctx_piece(st0, st1, 1, 1, close=True),
             lambda: ctx_piece(st0, st1, 2, 0),
             lambda: ctx_piece(st0, st1, 2, 1),
             lambda: (ctx_piece(st0, st1, 2, 2, close=True),
                      ctx_piece(st0, st1, 3, 0),
                      ctx_piece(st0, st1, 3, 1)),
             lambda: ctx_piece(st0, st1, 3, 2),
             lambda: ctx_piece(st0, st1, 3, 3, close=True),
             lambda: ctx_piece(st2, st3, 2, 0),
             lambda: ctx_piece(st2, st3, 2, 1),
             lambda: ctx_piece(st2, st3, 3, 0),
             lambda: ctx_piece(st2, st3, 3, 1),
             lambda: ctx_piece(st2, st3, 3, 2)]
    sched1 = {0: 1, 1: 1, 2: 1, 3: 1, 4: 1, 5: 1, 6: 1, 7: 1,
              8: 0, 9: 1, 10: 1, 11: 0, 12: 1, 13: 1, 14: 1, 15: 0}
    post1 = {3: [lambda: z_v2(st2, st3, 0),
                 lambda: ctx_piece(st2, st3, 0, 0, close=True)],
             7: [lambda: z_v2(st2, st3, 1),
                 lambda: ctx_piece(st2, st3, 1, 0),
                 lambda: ctx_piece(st2, st3, 1, 1, close=True)],
             11: [lambda: z_v2(st2, st3, 2),
                  lambda: ctx_piece(st2, st3, 2, 2, close=True)],
             15: [lambda: z_v2(st2, st3, 3),
                  lambda: ctx_piece(st2, st3, 3, 3, close=True)]}
    ROWS1 = list(range(KT))
    run_rows(st2, st3, ROWS1, sched1, fill1, post1)


_PROG = None


def _build_program():
    global _PROG
    if _PROG is not None:
        return _PROG
    nc = bacc.Bacc("TRN2", target_bir_lowering=False, debug=False,
                   num_devices=NCORES)
    xT = nc.dram_tensor("xT", [128, 4 * KC * 512], BF16,
                        kind="ExternalInput").ap()
    wq = nc.dram_tensor("wq", [128, 2 * KC * 128], BF16,
                        kind="ExternalInput").ap()
    wk = nc.dram_tensor("wk", [128, 2 * KC * 128], BF16,
                        kind="ExternalInput").ap()
    wv = nc.dram_tensor("wv", [128, KC * HL * HD], BF16,
                        kind="ExternalInput").ap()
    out = nc.dram_tensor("out", [HL * HD, S], BF16, kind="ExternalOutput").ap()
    with tile.TileContext(nc) as tc:
        with ExitStack() as stack:
            _emit(stack, tc, out, xT, wq, wk, wv)
    nc.compile()
    _PROG = nc
    return nc


def _pack_x(xb):
    # x[b] [S, D] -> xT [D, S] -> [128, sc, c, 512]: row p holds, per
    # 512-query chunk sc, all KC contraction chunks contiguously.
    xT = np.asarray(xb).T                      # [D, S] = [c*128+p, s]
    t = xT.reshape(KC, 128, 4, 512)            # [c, p, sc, s]
    return np.ascontiguousarray(
        t.transpose(1, 2, 0, 3).reshape(128, 4 * KC * 512))


def _pack_wqk(W):
    # W [D, 256] -> [128, pair, c, 128]
    t = np.asarray(W).reshape(KC, 128, 2, 128)  # [c, p, pair, n]
    return np.ascontiguousarray(
        t.transpose(1, 2, 0, 3).reshape(128, 2 * KC * 128))


def _pack_wv(W):
    # W [D, 256] -> [128, c, 256]
    t = np.asarray(W).reshape(KC, 128, HL * HD)  # [c, p, n]
    return np.ascontiguousarray(
        t.transpose(1, 0, 2).reshape(128, KC * HL * HD))


def make_in_maps(x, Wq, Wk, Wv):
    bf = ml_dtypes.bfloat16
    in_maps = []
    for core in range(NCORES):
        b, g = divmod(core, NCORES // B)
        cols = slice(HL * HD * g, HL * HD * (g + 1))
        in_maps.append({
            "xT": _pack_x(x[b]).astype(bf),
            "wq": _pack_wqk(np.asarray(Wq)[:, cols]).astype(bf),
            "wk": _pack_wqk(np.asarray(Wk)[:, cols]).astype(bf),
            "wv": _pack_wv(np.asarray(Wv)[:, cols]).astype(bf),
        })
    return in_maps


def assemble(results):
    out = np.empty((B, S, H * HD), np.float32)
    for core in range(NCORES):
        b, g = divmod(core, NCORES // B)
        out[b, :, HL * HD * g:HL * HD * (g + 1)] = \
            results[core]["out"].astype(np.float32).T
    return out


def kernel(**inputs):
    nc = _build_program()
    in_maps = make_in_maps(inputs["x"], inputs["Wq"], inputs["Wk"], inputs["Wv"])
    res = run_bass_kernel_spmd(nc, in_maps, list(range(NCORES)))
    return assemble(res.results)
